# revision 20
# baseline (speedup 1.0000x reference)
"""Trainium2 Bass kernel for nn_Encoder_46943992545741 (gnn_message_passing).

Math (see reference):
  uw = cumsum(u_weight, 0); vw = cumsum(v_weight, 0)
  tmp_u[r,n,h] = u_feat[n,:] @ uw[r]     tmp_v[r,m,h] = v_feat[m,:] @ vw[r]
  row[r,n] = sum_m support[r,n,m]        col[r,m] = sum_n support[r,n,m]
  sn[r,n,m] = rsqrt(row)[r,n] * support[r,n,m] * rsqrt(col)[r,m]
  ZU[n,h] = sum_r sum_m sn[r,n,m] * tmp_v[r,m,h]
  ZV[m,h] = sum_r sum_n sn[r,n,m] * tmp_u[r,n,h]
  z_u = relu(ZU[u] + bias); z_v = relu(ZV[v] + bias)

Distribution (zero-collective): core c owns n-shard c for the V side and
m-chunk c for the U side.  The host prep normalizes support once (sn, exact
f32 degree scalings folded into the bf16 cast it already performs) so the
device streams sn[:, nsh_c, :] (natural) and sn[:, :, msh_c]^T (transposed
chunk), both [5, 512, 4096] bf16, through a pure DMA->PE pipeline:
  - natural rows: stationary tmp_u -> partial-over-n ZV for ALL m,
  - transposed cols: stationary tmp_v -> partial-over-m ZU for ALL n,
with the r-sum accumulated on-chip (PSUM per relation, summed into an SBUF
f32 accumulator by VE/ACT), so each core outputs just 2x[64,4096] bf16.
No cross-core communication: the host sums the 8 per-core partials,
index-gathers, adds bias and relu (O(B*H) glue).  Per core ~44MB of DMA at
the ~358GB/s HBM-per-core roofline.
"""

import numpy as np
import ml_dtypes
from contextlib import ExitStack

import concourse.bacc as bacc
import concourse.mybir as mybir
import concourse.tile as tile
from concourse.bass_utils import run_bass_kernel_spmd

BF16 = mybir.dt.bfloat16
F32 = mybir.dt.float32
ADD = mybir.AluOpType.add

NCORES = 8
NU = 4096
NV = 4096
D = 256
H = 64
R = 5


def build_program(ncores=NCORES, nu=NU, nv=NV, d=D, h=H, r=R, repeat=1):
    nsh = nu // ncores           # rows / cols owned per core
    sbc = nsh // 128             # 128-strips per relation (4)
    dbc = d // 128               # contraction blocks for feature matmuls
    wid = nv                     # moving width per relation (4096)
    qpw = 1024                   # psum tile width (2 banks)
    qpc = wid // qpw             # psum tiles per relation-side (4)
    rh = r * h

    nc = bacc.Bacc()
    sup_n = nc.dram_tensor("sup_n", [r, nsh, nv], BF16, kind="ExternalInput")
    sup_t = nc.dram_tensor("sup_t", [r, nsh, nu], BF16, kind="ExternalInput")
    ufT = nc.dram_tensor("ufT", [dbc, 128, nsh], BF16, kind="ExternalInput")
    vfT = nc.dram_tensor("vfT", [dbc, 128, nsh], BF16, kind="ExternalInput")
    uwt = nc.dram_tensor("uwt", [dbc, 128, rh], BF16, kind="ExternalInput")
    vwt = nc.dram_tensor("vwt", [dbc, 128, rh], BF16, kind="ExternalInput")
    zu_p = nc.dram_tensor("zu_p", [h, nu], BF16, kind="ExternalOutput")
    zv_p = nc.dram_tensor("zv_p", [h, nv], BF16, kind="ExternalOutput")

    with tile.TileContext(nc) as tc, ExitStack() as ctx:
        wpool = ctx.enter_context(tc.tile_pool(name="weights", bufs=1))
        tmp = ctx.enter_context(tc.tile_pool(name="tmp", bufs=1))
        stm_n = ctx.enter_context(tc.tile_pool(name="stm_n", bufs=6))
        stm_t = ctx.enter_context(tc.tile_pool(name="stm_t", bufs=6))
        stage = ctx.enter_context(tc.tile_pool(name="stage", bufs=8))

        ufT_sb = wpool.tile([128, dbc, nsh], BF16)
        vfT_sb = wpool.tile([128, dbc, nsh], BF16)
        uw_sb = wpool.tile([128, dbc, rh], BF16)
        vw_sb = wpool.tile([128, dbc, rh], BF16)
        tmpu_sb = tmp.tile([128, sbc, rh], BF16)    # [p, strip, r*h]
        tmpv_sb = tmp.tile([128, sbc, rh], BF16)
        acc_u = tmp.tile([h, wid], F32)             # r-summed ZU partial
        acc_v = tmp.tile([h, wid], F32)
        # small loads go on the scalar queue so support streaming owns the
        # sync queue from t=0
        for db in range(dbc):
            nc.scalar.dma_start(ufT_sb[:, db, :], ufT[db])
            nc.scalar.dma_start(vfT_sb[:, db, :], vfT[db])
            nc.scalar.dma_start(uw_sb[:, db, :], uwt[db])
            nc.scalar.dma_start(vw_sb[:, db, :], vwt[db])

        for _rep in range(repeat):
            # ---- phase 0: tmp_u / tmp_v (feature x cumsum-weight matmuls) ----
            with tc.tile_pool(name="psum0", bufs=2, space="PSUM") as psum0:
                for fT, w, dst in ((ufT_sb, uw_sb, tmpu_sb),
                                   (vfT_sb, vw_sb, tmpv_sb)):
                    for s in range(sbc):
                        p0 = psum0.tile([128, rh], F32)
                        for db in range(dbc):
                            nc.tensor.matmul(
                                p0[:], fT[:, db, s * 128:(s + 1) * 128],
                                w[:, db, :], start=(db == 0), stop=(db == dbc - 1))
                        nc.vector.tensor_copy(dst[:, s, :], p0[:])

            # ---- stream both orientations, all relations, no collectives ----
            # The r-sum accumulates in acc_u/acc_v: relation 0 copies PSUM
            # into the accumulator, later relations add.  Each psum tile's
            # drain is deferred into the next segment so VE/ACT never
            # idle-wait on the PE; engines are fixed per psum column so the
            # read-modify-write chain per region stays in-order.
            with tc.tile_pool(name="psum", bufs=4, space="PSUM") as psum:
                def drain(pend):
                    acc_, rr_, pqs = pend
                    for q, pq in zip(range(qpc), pqs, strict=True):
                        dst = acc_[:, q * qpw:(q + 1) * qpw]
                        if rr_ == 0:
                            nc.vector.tensor_copy(dst, pq[:])
                        else:
                            nc.vector.tensor_tensor(dst, dst, pq[:], op=ADD)

                pending = None
                segs = [(rr, sup, spool, tmp_sb, acc)
                        for rr in range(r)
                        for sup, spool, tmp_sb, acc in (
                            (sup_t, stm_t, tmpv_sb, acc_u),
                            (sup_n, stm_n, tmpu_sb, acc_v))]
                for si, (rr, sup, spool, tmp_sb, acc) in enumerate(segs):
                    strips = []
                    for s in range(sbc):
                        st = spool.tile([128, wid], BF16, name="stm",
                                        tag=spool.name)
                        nc.sync.dma_start(
                            st[:], sup[rr, s * 128:(s + 1) * 128, :])
                        strips.append(st)
                    if pending is not None:
                        drain(pending)
                    pqs = []
                    for q in range(qpc):
                        pq = psum.tile([h, qpw], F32, name="pq", tag="pq")
                        for s in range(sbc):
                            off = q * qpw
                            nc.tensor.matmul(
                                pq[:, 0:512], tmp_sb[:, s, rr * h:(rr + 1) * h],
                                strips[s][:, off:off + 512],
                                start=(s == 0), stop=(s == sbc - 1))
                            nc.tensor.matmul(
                                pq[:, 512:qpw], tmp_sb[:, s, rr * h:(rr + 1) * h],
                                strips[s][:, off + 512:off + qpw],
                                start=(s == 0), stop=(s == sbc - 1))
                        pqs.append(pq)
                    pending = (acc, rr, pqs)
                drain(pending)

            # final cast + writeback (tiny: 2 x 0.5MB)
            for acc, out in ((acc_u, zu_p), (acc_v, zv_p)):
                for q in range(qpc):
                    stg = stage.tile([h, qpw], BF16, name="stg", tag="stg")
                    if q % 2 == 0:
                        nc.vector.tensor_copy(stg[:], acc[:, q * qpw:(q + 1) * qpw])
                    else:
                        nc.scalar.copy(stg[:], acc[:, q * qpw:(q + 1) * qpw])
                    nc.scalar.dma_start(out[:, q * qpw:(q + 1) * qpw], stg[:])
    nc.finalize()
    return nc


def prep_inputs(u_feat, v_feat, support, u_weight, v_weight, ncores=NCORES):
    """Host-side sharding / layout prep.  Returns per-core input dicts."""
    bf = ml_dtypes.bfloat16
    r, nu, nv = support.shape
    d, h = u_weight.shape[1], u_weight.shape[2]
    dbc = d // 128
    nsh = nu // ncores

    # symmetric degree normalization folded into the bf16 cast
    col = support.sum(axis=1)                 # [r, nv] (sum over n)
    row = support.sum(axis=2)                 # [r, nu] (sum over m)
    rinv = np.where(col > 0, 1.0 / np.sqrt(np.where(col > 0, col, 1.0)), 0.0)
    cinv = np.where(row > 0, 1.0 / np.sqrt(np.where(row > 0, row, 1.0)), 0.0)
    sn = support * cinv[:, :, None].astype(np.float32)
    sn *= rinv[:, None, :].astype(np.float32)

    sup16 = sn.astype(bf)
    supT16 = np.ascontiguousarray(sup16.transpose(0, 2, 1))
    uw = np.cumsum(u_weight.astype(np.float32), axis=0)
    vw = np.cumsum(v_weight.astype(np.float32), axis=0)

    def wt(w):  # [r, d, h] -> [dbc, 128, r*h]
        return np.ascontiguousarray(
            w.reshape(r, dbc, 128, h).transpose(1, 2, 0, 3)
            .reshape(dbc, 128, r * h)).astype(bf)

    ufT = np.ascontiguousarray(u_feat.T).astype(bf)       # [d, nu]
    vfT = np.ascontiguousarray(v_feat.T).astype(bf)       # [d, nv]
    uwt_d, vwt_d = wt(uw), wt(vw)

    in_maps = []
    for c in range(ncores):
        sl = slice(c * nsh, (c + 1) * nsh)
        in_maps.append({
            "sup_n": np.ascontiguousarray(sup16[:, sl, :]),
            "sup_t": np.ascontiguousarray(supT16[:, sl, :]),
            "ufT": np.ascontiguousarray(ufT[:, sl]).reshape(dbc, 128, nsh),
            "vfT": np.ascontiguousarray(vfT[:, sl]).reshape(dbc, 128, nsh),
            "uwt": uwt_d,
            "vwt": vwt_d,
        })
    return in_maps


def postprocess(results, u, v, u_bias, ncores=NCORES):
    """Combine per-core partials into (relu(z_u), relu(z_v))."""
    ZU = sum(results[c]["zu_p"].astype(np.float64) for c in range(ncores)).T
    ZV = sum(results[c]["zv_p"].astype(np.float64) for c in range(ncores)).T
    bias = np.asarray(u_bias, np.float64)
    zu = np.maximum(ZU[np.asarray(u)] + bias, 0.0).astype(np.float32)
    zv = np.maximum(ZV[np.asarray(v)] + bias, 0.0).astype(np.float32)
    return zu, zv


_PROGRAM = None


def kernel(u_feat, v_feat, u, v, support, u_weight, v_weight, u_bias,
           **run_kwargs):
    global _PROGRAM
    u_feat = np.asarray(u_feat, np.float32)
    v_feat = np.asarray(v_feat, np.float32)
    support = np.asarray(support, np.float32)
    u_weight = np.asarray(u_weight, np.float32)
    v_weight = np.asarray(v_weight, np.float32)
    u = np.asarray(u)
    v = np.asarray(v)

    if _PROGRAM is None:
        _PROGRAM = build_program()
    in_maps = prep_inputs(u_feat, v_feat, support, u_weight, v_weight)
    res = run_bass_kernel_spmd(
        _PROGRAM, in_maps, core_ids=list(range(NCORES)), **run_kwargs)
    return postprocess(res.results, u, v, np.asarray(u_bias, np.float32))


# revision 23
# speedup vs baseline: 1.0223x; 1.0223x over previous
"""Trainium2 Bass kernel for nn_Encoder_46943992545741 (gnn_message_passing).

Math (see reference):
  uw = cumsum(u_weight, 0); vw = cumsum(v_weight, 0)
  tmp_u[r,n,h] = u_feat[n,:] @ uw[r]     tmp_v[r,m,h] = v_feat[m,:] @ vw[r]
  row[r,n] = sum_m support[r,n,m]        col[r,m] = sum_n support[r,n,m]
  sn[r,n,m] = rsqrt(row)[r,n] * support[r,n,m] * rsqrt(col)[r,m]
  ZU[n,h] = sum_r sum_m sn[r,n,m] * tmp_v[r,m,h]
  ZV[m,h] = sum_r sum_n sn[r,n,m] * tmp_u[r,n,h]
  z_u = relu(ZU[u] + bias); z_v = relu(ZV[v] + bias)

Distribution (zero-collective): core c owns n-shard c for the V side and
m-chunk c for the U side.  The host prep normalizes support once (sn, exact
f32 degree scalings folded into the bf16 cast it already performs) so the
device streams sn[:, nsh_c, :] (natural) and sn[:, :, msh_c]^T (transposed
chunk), both [5, 512, 4096] bf16, through a pure DMA->PE pipeline:
  - natural rows: stationary tmp_u -> partial-over-n ZV for ALL m,
  - transposed cols: stationary tmp_v -> partial-over-m ZU for ALL n,
with the r-sum accumulated on-chip (PSUM per relation, summed into an SBUF
f32 accumulator by VE/ACT), so each core outputs just 2x[64,4096] bf16.
No cross-core communication: the host sums the 8 per-core partials,
index-gathers, adds bias and relu (O(B*H) glue).  Per core ~44MB of DMA at
the ~358GB/s HBM-per-core roofline.
"""

import numpy as np
import ml_dtypes
from contextlib import ExitStack

import concourse.bacc as bacc
import concourse.mybir as mybir
import concourse.tile as tile
from concourse.bass_utils import run_bass_kernel_spmd

BF16 = mybir.dt.bfloat16
F32 = mybir.dt.float32
ADD = mybir.AluOpType.add

NCORES = 8
NU = 4096
NV = 4096
D = 256
H = 64
R = 5


def build_program(ncores=NCORES, nu=NU, nv=NV, d=D, h=H, r=R, repeat=1):
    nsh = nu // ncores           # rows / cols owned per core
    sbc = nsh // 128             # 128-strips per relation (4)
    dbc = d // 128               # contraction blocks for feature matmuls
    wid = nv                     # moving width per relation (4096)
    qpw = 1024                   # psum tile width (2 banks)
    qpc = wid // qpw             # psum tiles per relation-side (4)
    rh = r * h

    nc = bacc.Bacc()
    sup_n = nc.dram_tensor("sup_n", [r, nsh, nv], BF16, kind="ExternalInput")
    sup_t = nc.dram_tensor("sup_t", [r, nsh, nu], BF16, kind="ExternalInput")
    ufT = nc.dram_tensor("ufT", [dbc, 128, nsh], BF16, kind="ExternalInput")
    vfT = nc.dram_tensor("vfT", [dbc, 128, nsh], BF16, kind="ExternalInput")
    uwt = nc.dram_tensor("uwt", [dbc, 128, rh], BF16, kind="ExternalInput")
    vwt = nc.dram_tensor("vwt", [dbc, 128, rh], BF16, kind="ExternalInput")
    zu_p = nc.dram_tensor("zu_p", [h, nu], BF16, kind="ExternalOutput")
    zv_p = nc.dram_tensor("zv_p", [h, nv], BF16, kind="ExternalOutput")

    with tile.TileContext(nc) as tc, ExitStack() as ctx:
        wpool = ctx.enter_context(tc.tile_pool(name="weights", bufs=1))
        tmp = ctx.enter_context(tc.tile_pool(name="tmp", bufs=1))
        stm_n = ctx.enter_context(tc.tile_pool(name="stm_n", bufs=6))
        stm_t = ctx.enter_context(tc.tile_pool(name="stm_t", bufs=6))
        stage = ctx.enter_context(tc.tile_pool(name="stage", bufs=8))

        ufT_sb = wpool.tile([128, dbc, nsh], BF16)
        vfT_sb = wpool.tile([128, dbc, nsh], BF16)
        uw_sb = wpool.tile([128, dbc, rh], BF16)
        vw_sb = wpool.tile([128, dbc, rh], BF16)
        tmpu_sb = tmp.tile([128, sbc, rh], BF16)    # [p, strip, r*h]
        tmpv_sb = tmp.tile([128, sbc, rh], BF16)
        acc_u = tmp.tile([h, wid], F32)             # r-summed ZU partial
        acc_v = tmp.tile([h, wid], F32)
        # small loads go on the scalar queue so support streaming owns the
        # sync queue from t=0
        for db in range(dbc):
            nc.scalar.dma_start(ufT_sb[:, db, :], ufT[db])
            nc.scalar.dma_start(vfT_sb[:, db, :], vfT[db])
            nc.scalar.dma_start(uw_sb[:, db, :], uwt[db])
            nc.scalar.dma_start(vw_sb[:, db, :], vwt[db])

        for _rep in range(repeat):
            # ---- phase 0: tmp_u / tmp_v (feature x cumsum-weight matmuls) ----
            with tc.tile_pool(name="psum0", bufs=2, space="PSUM") as psum0:
                for fT, w, dst in ((ufT_sb, uw_sb, tmpu_sb),
                                   (vfT_sb, vw_sb, tmpv_sb)):
                    for s in range(sbc):
                        p0 = psum0.tile([128, rh], F32)
                        for db in range(dbc):
                            nc.tensor.matmul(
                                p0[:], fT[:, db, s * 128:(s + 1) * 128],
                                w[:, db, :], start=(db == 0), stop=(db == dbc - 1))
                        nc.vector.tensor_copy(dst[:, s, :], p0[:])

            # ---- stream both orientations, all relations, no collectives ----
            # The r-sum accumulates in acc_u/acc_v: relation 0 copies PSUM
            # into the accumulator, later relations add.  Each psum tile's
            # drain is deferred into the next segment so VE/ACT never
            # idle-wait on the PE; engines are fixed per psum column so the
            # read-modify-write chain per region stays in-order.
            with tc.tile_pool(name="psum", bufs=4, space="PSUM") as psum:
                def writeback(acc_, out_):
                    # final cast + DMA out (tiny: 0.5MB per side)
                    for q in range(qpc):
                        stg = stage.tile([h, qpw], BF16, name="stg", tag="stg")
                        sl = slice(q * qpw, (q + 1) * qpw)
                        if q % 2 == 0:
                            nc.vector.tensor_copy(stg[:], acc_[:, sl])
                        else:
                            nc.scalar.copy(stg[:], acc_[:, sl])
                        nc.scalar.dma_start(out_[:, sl], stg[:])

                def drain(pend):
                    acc_, rr_, pqs, out_ = pend
                    for q, pq in zip(range(qpc), pqs, strict=True):
                        dst = acc_[:, q * qpw:(q + 1) * qpw]
                        if rr_ == 0:
                            nc.vector.tensor_copy(dst, pq[:])
                        else:
                            nc.vector.tensor_tensor(dst, dst, pq[:], op=ADD)
                    if rr_ == r - 1:
                        writeback(acc_, out_)

                pending = None
                segs = [(rr, sup, spool, tmp_sb, acc, out)
                        for rr in range(r)
                        for sup, spool, tmp_sb, acc, out in (
                            (sup_t, stm_t, tmpv_sb, acc_u, zu_p),
                            (sup_n, stm_n, tmpu_sb, acc_v, zv_p))]
                for si, (rr, sup, spool, tmp_sb, acc, out) in enumerate(segs):
                    strips = []
                    for s in range(sbc):
                        st = spool.tile([128, wid], BF16, name="stm",
                                        tag=spool.name)
                        nc.sync.dma_start(
                            st[:], sup[rr, s * 128:(s + 1) * 128, :])
                        strips.append(st)
                    if pending is not None:
                        drain(pending)
                    pqs = []
                    for q in range(qpc):
                        pq = psum.tile([h, qpw], F32, name="pq", tag="pq")
                        for s in range(sbc):
                            off = q * qpw
                            nc.tensor.matmul(
                                pq[:, 0:512], tmp_sb[:, s, rr * h:(rr + 1) * h],
                                strips[s][:, off:off + 512],
                                start=(s == 0), stop=(s == sbc - 1))
                            nc.tensor.matmul(
                                pq[:, 512:qpw], tmp_sb[:, s, rr * h:(rr + 1) * h],
                                strips[s][:, off + 512:off + qpw],
                                start=(s == 0), stop=(s == sbc - 1))
                        pqs.append(pq)
                    pending = (acc, rr, pqs, out)
                drain(pending)

    nc.finalize()
    return nc


def prep_inputs(u_feat, v_feat, support, u_weight, v_weight, ncores=NCORES):
    """Host-side sharding / layout prep.  Returns per-core input dicts."""
    bf = ml_dtypes.bfloat16
    r, nu, nv = support.shape
    d, h = u_weight.shape[1], u_weight.shape[2]
    dbc = d // 128
    nsh = nu // ncores

    # symmetric degree normalization folded into the bf16 cast
    col = support.sum(axis=1)                 # [r, nv] (sum over n)
    row = support.sum(axis=2)                 # [r, nu] (sum over m)
    rinv = np.where(col > 0, 1.0 / np.sqrt(np.where(col > 0, col, 1.0)), 0.0)
    cinv = np.where(row > 0, 1.0 / np.sqrt(np.where(row > 0, row, 1.0)), 0.0)
    sn = support * cinv[:, :, None].astype(np.float32)
    sn *= rinv[:, None, :].astype(np.float32)

    sup16 = sn.astype(bf)
    supT16 = np.ascontiguousarray(sup16.transpose(0, 2, 1))
    uw = np.cumsum(u_weight.astype(np.float32), axis=0)
    vw = np.cumsum(v_weight.astype(np.float32), axis=0)

    def wt(w):  # [r, d, h] -> [dbc, 128, r*h]
        return np.ascontiguousarray(
            w.reshape(r, dbc, 128, h).transpose(1, 2, 0, 3)
            .reshape(dbc, 128, r * h)).astype(bf)

    ufT = np.ascontiguousarray(u_feat.T).astype(bf)       # [d, nu]
    vfT = np.ascontiguousarray(v_feat.T).astype(bf)       # [d, nv]
    uwt_d, vwt_d = wt(uw), wt(vw)

    in_maps = []
    for c in range(ncores):
        sl = slice(c * nsh, (c + 1) * nsh)
        in_maps.append({
            "sup_n": np.ascontiguousarray(sup16[:, sl, :]),
            "sup_t": np.ascontiguousarray(supT16[:, sl, :]),
            "ufT": np.ascontiguousarray(ufT[:, sl]).reshape(dbc, 128, nsh),
            "vfT": np.ascontiguousarray(vfT[:, sl]).reshape(dbc, 128, nsh),
            "uwt": uwt_d,
            "vwt": vwt_d,
        })
    return in_maps


def postprocess(results, u, v, u_bias, ncores=NCORES):
    """Combine per-core partials into (relu(z_u), relu(z_v))."""
    ZU = sum(results[c]["zu_p"].astype(np.float64) for c in range(ncores)).T
    ZV = sum(results[c]["zv_p"].astype(np.float64) for c in range(ncores)).T
    bias = np.asarray(u_bias, np.float64)
    zu = np.maximum(ZU[np.asarray(u)] + bias, 0.0).astype(np.float32)
    zv = np.maximum(ZV[np.asarray(v)] + bias, 0.0).astype(np.float32)
    return zu, zv


_PROGRAM = None


def kernel(u_feat, v_feat, u, v, support, u_weight, v_weight, u_bias,
           **run_kwargs):
    global _PROGRAM
    u_feat = np.asarray(u_feat, np.float32)
    v_feat = np.asarray(v_feat, np.float32)
    support = np.asarray(support, np.float32)
    u_weight = np.asarray(u_weight, np.float32)
    v_weight = np.asarray(v_weight, np.float32)
    u = np.asarray(u)
    v = np.asarray(v)

    if _PROGRAM is None:
        _PROGRAM = build_program()
    in_maps = prep_inputs(u_feat, v_feat, support, u_weight, v_weight)
    last_err = None
    for _attempt in range(3):   # transient NRT device errors: retry
        try:
            res = run_bass_kernel_spmd(
                _PROGRAM, in_maps, core_ids=list(range(NCORES)), **run_kwargs)
            break
        except Exception as e:  # noqa: BLE001
            last_err = e
    else:
        raise last_err
    return postprocess(res.results, u, v, np.asarray(u_bias, np.float32))
